# revision 1
# baseline (speedup 1.0000x reference)
# GraphSAGE mean-aggregation layer on 8 Trainium2 NeuronCores.
#
# Sharding: destination nodes are partitioned across the 8 cores (6250 each).
# Each core receives the full node-feature table x (for gathers), plus
# host-packed per-core metadata:
#   - its edges, grouped by 128-destination "blocks", padded to 128-edge tiles
#   - int16 gather indices (dma_gather requires int16, so edges are split into
#     src<32768 ("lo") and src>=32768 ("hi") groups gathered from two base
#     offsets of x)
#   - per-edge destination-within-block ids (f32, sentinel 999 for padding)
# Device per block of 128 destinations:
#   agg[dst, feat]  = sum over edge tiles of S_t.T @ msgs_t   (PE, PSUM accum)
#   cnt[dst]        = sum over edge tiles of S_t.T @ ones     (PE)
#     where S_t[e, d] = (rel[e] == d) is built on DVE via is_equal vs an iota
#     row; padded edges have rel=999 so they contribute nothing.
#   mean = agg * 1/max(cnt,1)                                  (DVE)
#   out  = mean @ W_l.T + x_dst @ W_r.T + b_l                  (PE; x_dst
#     supplied pre-transposed from host, mean transposed on PE)
#   y    = out / max(||out||_2, 1e-12)                         (ACT+DVE)
#
# The Bass program is identical across cores (capacities = max over cores),
# so it runs SPMD via run_bass_kernel_spmd; per-core data differs only in the
# input tensors.

import os
from contextlib import ExitStack

import numpy as np

import concourse.bacc as bacc
import concourse.mybir as mybir
import concourse.tile as tile
from concourse.bass_utils import run_bass_kernel_spmd
from concourse.masks import make_identity

F = 128          # feature dim (in_c == out_c == 128)
BLK = 128        # destinations per block (= PSUM partition dim)
N_NODES = 50000
N_EDGES = 800000
N_CORES = 8
HALF = 32768     # int16 gather-index limit
CHUNK_BLOCKS = 4   # dst blocks per gather chunk
ACT_SBUILD = (0, 1)  # build S on ACT for this fraction of tiles (DVE/ACT balance)
MSG_BF16 = True      # gather/aggregate messages in bf16 (GEMMs stay f32)
DMA_SCRATCH = 16384  # SWDGE descriptor-ring bytes/partition
GMAX_IDX = 1024      # indices per dma_gather call (HW ucode limit)
SENT = 512.0         # padded-edge rel sentinel (exact in bf16, >= 128)


def _make_plan(counts_max, chunk_blocks):
    """Shared (across cores) tile schedule from per-(block,half) max counts."""
    caps = -(-counts_max // BLK)  # ceil div -> tiles per (block, half)
    NB = caps.shape[0]
    chunks = []
    t0 = 0
    for k0 in range(0, NB, chunk_blocks):
        bl = list(range(k0, min(k0 + chunk_blocks, NB)))
        gcols = {}
        t = t0
        for h in (0, 1):
            for b in bl:
                if caps[b, h]:
                    gcols[(b, h)] = t
                    t += int(caps[b, h])
        chunks.append(
            dict(
                blocks=bl,
                gcols=gcols,
                start=t0,
                lo_tiles=int(sum(caps[b, 0] for b in bl)),
                hi_tiles=int(sum(caps[b, 1] for b in bl)),
            )
        )
        t0 = t
    return caps, chunks, t0


def _pack_inputs(x, src, dst, n_nodes, n_cores, half, chunk_blocks):
    """Host-side graph partitioning: bucket edges by (core, block, half),
    pad each bucket to whole 128-edge tiles, emit per-core device arrays."""
    NP = n_nodes // n_cores
    assert NP * n_cores == n_nodes
    NB = -(-NP // BLK)
    core = dst // NP
    ldst = dst - core * NP
    blk = ldst // BLK
    rel = (ldst - blk * BLK).astype(np.float32)
    halfv = (src >= half).astype(np.int64)
    gkey = (core * NB + blk) * 2 + halfv
    counts = np.bincount(gkey, minlength=n_cores * NB * 2).reshape(n_cores, NB, 2)
    caps, chunks, T_total = _make_plan(counts.max(axis=0), chunk_blocks)

    tile_col = np.zeros((NB, 2), np.int64)
    for ch in chunks:
        for (b, h), c in ch["gcols"].items():
            tile_col[b, h] = c

    order = np.argsort(gkey, kind="stable")
    gsorted = gkey[order]
    gstart = np.searchsorted(gsorted, np.arange(n_cores * NB * 2))
    rank = np.empty(len(gkey), np.int64)
    rank[order] = np.arange(len(gkey)) - gstart[gsorted]
    pos = tile_col[blk, halfv] * BLK + rank  # padded slot within the core
    idxval = np.where(halfv == 1, src - half, src).astype(np.int16)

    total_pad = T_total * BLK
    deg = np.bincount(dst, minlength=n_nodes).astype(np.float32)
    rdeg = 1.0 / np.maximum(deg, 1.0)
    per_core = []
    for c in range(n_cores):
        m = core == c
        idx_pad = np.zeros(total_pad, np.int16)
        rel_pad = np.full(total_pad, SENT, np.float32)
        idx_pad[pos[m]] = idxval[m]
        rel_pad[pos[m]] = rel[m]
        # dma_gather index layout: partition e%16, column e//16, replicated
        # across the eight 16-partition groups.
        idx_mat = np.ascontiguousarray(np.tile(idx_pad.reshape(-1, 16).T, (8, 1)))
        # gather output layout: partition e%128, tile-column e//128.
        rel_mat = np.ascontiguousarray(rel_pad.reshape(-1, BLK).T)
        xT = np.zeros((F, NB * BLK), np.float32)
        xT[:, :NP] = x[c * NP : (c + 1) * NP].T
        # 1/max(in-degree,1) for this core's dsts: [128, NB], column = block
        rc = np.zeros(NB * BLK, np.float32)
        rc[:NP] = rdeg[c * NP : (c + 1) * NP]
        rcnt_mat = np.ascontiguousarray(rc.reshape(NB, BLK).T)
        per_core.append((idx_mat, rel_mat, xT, rcnt_mat))
    return caps, chunks, T_total, NP, NB, per_core


def _build_program(caps, chunks, T_total, NP, NB, n_nodes, half, ablate=(),
                   repeat=1):
    dt = mybir.dt
    mdt = dt.bfloat16 if MSG_BF16 else dt.float32
    nc = bacc.Bacc(
        "TRN2", target_bir_lowering=False, debug=False,
        dynamic_dma_scratch_size=DMA_SCRATCH,
    )

    x_d = nc.dram_tensor("x", [n_nodes, F], mdt, kind="ExternalInput")
    xT_d = nc.dram_tensor("xT", [F, NB * BLK], dt.float32, kind="ExternalInput")
    idx_d = nc.dram_tensor("idx", [128, T_total * 8], dt.int16, kind="ExternalInput")
    rel_d = nc.dram_tensor("rel", [128, T_total], dt.float32, kind="ExternalInput")
    wlT_d = nc.dram_tensor("wlT", [F, F], dt.float32, kind="ExternalInput")
    wrT_d = nc.dram_tensor("wrT", [F, F], dt.float32, kind="ExternalInput")
    bias_d = nc.dram_tensor("bias", [1, F], dt.float32, kind="ExternalInput")
    iota_d = nc.dram_tensor("iota", [128, 128], mdt, kind="ExternalInput")
    rcnt_d = nc.dram_tensor("rcnt", [128, NB], dt.float32, kind="ExternalInput")
    y_d = nc.dram_tensor("y", [NP, F], dt.float32, kind="ExternalOutput")

    TCMAX = max(ch["lo_tiles"] + ch["hi_tiles"] for ch in chunks)

    with tile.TileContext(nc) as tc, ExitStack() as ctx:
        res = ctx.enter_context(tc.tile_pool(name="res", bufs=1))
        msgs_p = ctx.enter_context(tc.tile_pool(name="msgs", bufs=2))
        s_p = ctx.enter_context(tc.tile_pool(name="sel", bufs=4))
        work_p = ctx.enter_context(tc.tile_pool(name="work", bufs=3))
        small_p = ctx.enter_context(tc.tile_pool(name="small", bufs=4))
        agg_p = ctx.enter_context(tc.tile_pool(name="agg", bufs=3, space="PSUM"))
        pt_p = ctx.enter_context(tc.tile_pool(name="pt", bufs=2, space="PSUM"))
        po_p = ctx.enter_context(tc.tile_pool(name="po", bufs=2, space="PSUM"))

        xT_sb = res.tile([F, NB * BLK], dt.float32)
        nc.sync.dma_start(xT_sb[:], xT_d[:])
        rel_sb = res.tile([128, T_total], dt.float32)
        nc.sync.dma_start(rel_sb[:], rel_d[:])
        idx_sb = res.tile([128, T_total * 8], dt.int16)
        nc.sync.dma_start(idx_sb[:], idx_d[:])
        iota_sb = res.tile([128, 128], mdt)
        nc.sync.dma_start(iota_sb[:], iota_d[:])
        rcnt_sb = res.tile([128, NB], dt.float32)
        nc.sync.dma_start(rcnt_sb[:], rcnt_d[:])
        wlT_sb = res.tile([F, F], dt.float32)
        nc.sync.dma_start(wlT_sb[:], wlT_d[:])
        wrT_sb = res.tile([F, F], dt.float32)
        nc.sync.dma_start(wrT_sb[:], wrT_d[:])
        bias_sb = res.tile([1, F], dt.float32)
        nc.sync.dma_start(bias_sb[:], bias_d[:])
        ident_sb = res.tile([128, 128], dt.float32)
        make_identity(nc, ident_sb[:])
        ones_sb = res.tile([128, 1], dt.float32)
        nc.vector.memset(ones_sb[:], 1.0)
        onesrow_sb = res.tile([1, 128], dt.float32)
        nc.vector.memset(onesrow_sb[:], 1.0)

        for _rep in range(repeat):
          for ch in chunks:
            msgs = msgs_p.tile([128, TCMAX, F], mdt, tag="msgs")
            lo_t, hi_t = ch["lo_tiles"], ch["hi_tiles"]
            st = ch["start"]
            # Ring-capacity limit on indices per dma_gather call.
            GMAX = GMAX_IDX // BLK
            for base, count, src_ap in (
                (0, lo_t, x_d[0:half, :]),
                (lo_t, hi_t, x_d[half:n_nodes, :]),
            ):
                for g0 in range(0, count, GMAX):
                    gt = min(GMAX, count - g0)
                    if "gsmall" in ablate:
                        gt = 1  # timing-only ablation: 1/8 gather bytes
                    s0 = base + g0  # slot within chunk
                    nc.gpsimd.dma_gather(
                        out_ap=msgs[:, s0 : s0 + gt, :],
                        in_ap=src_ap,
                        idxs_ap=idx_sb[:, (st + s0) * 8 : (st + s0 + gt) * 8],
                        num_idxs=gt * BLK,
                        num_idxs_reg=gt * BLK,
                        elem_size=F,
                    )
            for b in ch["blocks"]:
                slots = []
                for h in (0, 1):
                    if caps[b, h]:
                        g0 = ch["gcols"][(b, h)]
                        slots.extend(range(g0 - st, g0 - st + int(caps[b, h])))
                nb = min(BLK, NP - b * BLK)
                psum_agg = agg_p.tile([128, F], dt.float32, tag="agg")
                if not slots or "seg" in ablate:
                    nc.vector.memset(psum_agg[:], 0.0)
                for j, slot in enumerate(slots):
                    if "seg" in ablate:
                        break
                    tcol = st + slot
                    S = s_p.tile([128, 128], mdt, tag="S")
                    if "sbuild" in ablate:
                        S = iota_sb
                    elif (tcol % ACT_SBUILD[1]) < ACT_SBUILD[0]:
                        # exact one-hot on ACT: |rel - iota| then relu(1 - | . |)
                        t1 = s_p.tile([128, 128], mdt, tag="Sa")
                        nc.scalar.activation(
                            out=t1[:], in_=iota_sb[:],
                            func=mybir.ActivationFunctionType.Abs,
                            bias=rel_sb[:, tcol : tcol + 1], scale=-1.0,
                        )
                        nc.scalar.activation(
                            out=S[:], in_=t1[:],
                            func=mybir.ActivationFunctionType.Relu,
                            bias=1.0, scale=-1.0,
                        )
                    else:
                        nc.vector.tensor_scalar(
                            out=S[:],
                            in0=iota_sb[:],
                            scalar1=rel_sb[:, tcol : tcol + 1],
                            scalar2=None,
                            op0=mybir.AluOpType.is_equal,
                        )
                    first, last = j == 0, j == len(slots) - 1
                    if "mm2" not in ablate:
                        nc.tensor.matmul(
                            psum_agg[:], lhsT=S[:], rhs=msgs[:, slot, :],
                            start=first, stop=last,
                        )
                    elif first:
                        nc.vector.memset(psum_agg[:], 0.0)
                mean = work_p.tile([128, F], dt.float32, tag="mean")
                nc.vector.tensor_scalar_mul(
                    mean[:], psum_agg[:], rcnt_sb[:, b : b + 1]
                )
                psum_t = pt_p.tile([128, 128], dt.float32, tag="pt")
                nc.tensor.transpose(psum_t[:], mean[:], ident_sb[:])
                mT = work_p.tile([128, 128], dt.float32, tag="mT")
                nc.vector.tensor_copy(out=mT[:], in_=psum_t[:])
                psum_o = po_p.tile([128, F], dt.float32, tag="po")
                nc.tensor.matmul(
                    psum_o[:], lhsT=mT[:], rhs=wlT_sb[:], start=True, stop=False
                )
                nc.tensor.matmul(
                    psum_o[:], lhsT=xT_sb[:, b * BLK : (b + 1) * BLK], rhs=wrT_sb[:],
                    start=False, stop=False,
                )
                nc.tensor.matmul(
                    psum_o[:], lhsT=onesrow_sb[0:1, :], rhs=bias_sb[0:1, :],
                    start=False, stop=True,
                )
                sq = work_p.tile([128, F], dt.float32, tag="sq")
                ss = small_p.tile([128, 1], dt.float32, tag="ss")
                nc.scalar.activation(
                    out=sq[:], in_=psum_o[:],
                    func=mybir.ActivationFunctionType.Square, accum_out=ss[:],
                )
                ssm = small_p.tile([128, 1], dt.float32, tag="ssm")
                nc.vector.tensor_scalar_max(ssm[:], ss[:], 1e-24)
                nrm = small_p.tile([128, 1], dt.float32, tag="nrm")
                nc.scalar.sqrt(nrm[:], ssm[:])
                rn = small_p.tile([128, 1], dt.float32, tag="rn")
                nc.vector.reciprocal(rn[:], nrm[:])
                outt = work_p.tile([128, F], dt.float32, tag="outt")
                nc.vector.tensor_scalar_mul(outt[:], psum_o[:], rn[:, 0:1])
                nc.sync.dma_start(y_d[b * BLK : b * BLK + nb, :], outt[0:nb, :])

    nc.compile()
    return nc


_CACHE = {}


def _prepare(inputs, n_nodes=N_NODES, n_cores=N_CORES, half=HALF,
             chunk_blocks=CHUNK_BLOCKS):
    import ml_dtypes
    mnp = ml_dtypes.bfloat16 if MSG_BF16 else np.float32
    x = np.asarray(inputs["x"], np.float32)
    ei = np.asarray(inputs["edge_index"], np.int64)
    W_l = np.asarray(inputs["W_l"], np.float32)
    b_l = np.asarray(inputs["b_l"], np.float32)
    W_r = np.asarray(inputs["W_r"], np.float32)
    src, dst = ei[0], ei[1]

    caps, chunks, T_total, NP, NB, per_core = _pack_inputs(
        x, src, dst, n_nodes, n_cores, half, chunk_blocks
    )
    key = (n_nodes, n_cores, half, chunk_blocks, caps.tobytes())
    nc = _CACHE.get(key)
    if nc is None:
        nc = _build_program(caps, chunks, T_total, NP, NB, n_nodes, half)
        _CACHE[key] = nc

    iota = np.ascontiguousarray(
        np.broadcast_to(np.arange(128), (128, 128)).astype(mnp)
    )
    x_m = np.ascontiguousarray(x.astype(mnp))
    wlT = np.ascontiguousarray(W_l.T)
    wrT = np.ascontiguousarray(W_r.T)
    bias = np.ascontiguousarray(b_l[None, :])
    in_maps = []
    for c in range(n_cores):
        idx_mat, rel_mat, xT, rcnt_mat = per_core[c]
        in_maps.append(
            {
                "x": x_m, "xT": xT, "idx": idx_mat, "rel": rel_mat,
                "wlT": wlT, "wrT": wrT, "bias": bias, "iota": iota,
                "rcnt": rcnt_mat,
            }
        )
    return nc, in_maps, NP


def _run(inputs, trace=False):
    nc, in_maps, NP = _prepare(inputs)
    r = run_bass_kernel_spmd(nc, in_maps, list(range(N_CORES)), trace=trace)
    y = np.concatenate([r.results[c]["y"] for c in range(N_CORES)], axis=0)
    return y, r


def kernel(**inputs) -> np.ndarray:
    y, _ = _run(inputs)
    return y



# revision 29
# speedup vs baseline: 4.2483x; 4.2483x over previous
# GraphSAGE mean-aggregation layer on 8 Trainium2 NeuronCores.
#
# Sharding: destination nodes are partitioned across the 8 cores (6250 each).
# Each core receives the full node-feature table x (for gathers), plus
# host-packed per-core metadata:
#   - its edges, grouped by 128-destination "blocks", padded to 128-edge tiles
#   - int16 gather indices (dma_gather requires int16, so edges are split into
#     src<32768 ("lo") and src>=32768 ("hi") groups gathered from two base
#     offsets of x)
#   - per-edge destination-within-block ids (f32, sentinel 999 for padding)
# Device per block of 128 destinations:
#   aggT[feat, dst] = sum over edge tiles of msgs_t.T @ S_t   (PE, PSUM accum)
#     where S_t[e, d] = (rel[e] == d) is built on DVE/ACT via is_equal vs an
#     iota row; padded edges have rel=SENT so they contribute nothing.
#   meanT = aggT * rcnt (1/max(in-degree,1), host-precomputed) (DVE)
#   out  = meanT.T @ W_l.T + x_dst @ W_r.T + b_l               (PE; meanT is
#     already the lhsT layout, x_dst supplied pre-transposed from host)
#   y    = out / max(||out||_2, 1e-12)                         (ACT+DVE)
# Gathers are issued round-robin on 2 SWDGE queues (descriptor generation for
# queue q runs on Q7 core pair 2q/2q+1; the Pool sequencer keeps 2 in flight),
# and each bucket's edges are sorted by src so gather reads ascend in HBM.
#
# The Bass program is identical across cores (capacities = max over cores),
# so it runs SPMD via run_bass_kernel_spmd; per-core data differs only in the
# input tensors.

import os
from contextlib import ExitStack

import numpy as np

import concourse.bacc as bacc
import concourse.mybir as mybir
import concourse.tile as tile
from concourse.bass_utils import run_bass_kernel_spmd

F = 128          # feature dim (in_c == out_c == 128)
BLK = 128        # destinations per block (= PSUM partition dim)
N_NODES = 50000
N_EDGES = 800000
N_CORES = 8
HALF = 32768     # int16 gather-index limit
CHUNK_BLOCKS = 8   # dst blocks per gather chunk
ACT_SBUILD = (0, 1)  # build S on ACT for this fraction of tiles (all-DVE wins)
MSG_BF16 = True      # gather/aggregate messages in bf16 (GEMMs stay f32)
SRC_SORT = False     # sort bucket edges by src (clusters HBM reads; slower)
MSGS_BUFS = 3        # msgs tile double/triple buffering
SINGLE_PACKET = False  # per-descriptor packets interleave queues better
BATCH_OUT = False    # per-block output DMAs beat the strided chunk DMA
DMA_SCRATCH = 16384  # SWDGE descriptor-ring bytes/partition
GMAX_IDX = 1024      # indices per dma_gather call (larger hangs: ring capacity)
N_QUEUES = 4         # SWDGE queues; queue q's descriptors are generated on Q7
                     # core pair 2q/2q+1, so round-robin parallelizes desc-gen
SENT = 512.0         # padded-edge rel sentinel (exact in bf16, >= 128)


def _make_plan(counts_max, chunk_blocks):
    """Shared (across cores) tile schedule from per-(block,half) max counts."""
    caps = -(-counts_max // BLK)  # ceil div -> tiles per (block, half)
    NB = caps.shape[0]
    chunks = []
    t0 = 0
    for k0 in range(0, NB, chunk_blocks):
        bl = list(range(k0, min(k0 + chunk_blocks, NB)))
        gcols = {}
        t = t0
        for h in (0, 1):
            for b in bl:
                if caps[b, h]:
                    gcols[(b, h)] = t
                    t += int(caps[b, h])
        chunks.append(
            dict(
                blocks=bl,
                gcols=gcols,
                start=t0,
                lo_tiles=int(sum(caps[b, 0] for b in bl)),
                hi_tiles=int(sum(caps[b, 1] for b in bl)),
            )
        )
        t0 = t
    return caps, chunks, t0


def _pack_inputs(x, src, dst, n_nodes, n_cores, half, chunk_blocks):
    """Host-side graph partitioning: bucket edges by (core, block, half),
    pad each bucket to whole 128-edge tiles, emit per-core device arrays."""
    NP = n_nodes // n_cores
    assert NP * n_cores == n_nodes
    NB = -(-NP // BLK)
    core = dst // NP
    ldst = dst - core * NP
    blk = ldst // BLK
    rel = (ldst - blk * BLK).astype(np.float32)
    halfv = (src >= half).astype(np.int64)
    gkey = (core * NB + blk) * 2 + halfv
    counts = np.bincount(gkey, minlength=n_cores * NB * 2).reshape(n_cores, NB, 2)
    caps, chunks, T_total = _make_plan(counts.max(axis=0), chunk_blocks)

    tile_col = np.zeros((NB, 2), np.int64)
    for ch in chunks:
        for (b, h), c in ch["gcols"].items():
            tile_col[b, h] = c

    # SRC_SORT orders each bucket's edges by src (ascending HBM addresses per
    # dma_gather call). Measured: clustering HURTS SDMA throughput on random
    # graphs, so default off.
    order = np.lexsort((src, gkey)) if SRC_SORT else np.argsort(gkey, kind="stable")
    gsorted = gkey[order]
    gstart = np.searchsorted(gsorted, np.arange(n_cores * NB * 2))
    rank = np.empty(len(gkey), np.int64)
    rank[order] = np.arange(len(gkey)) - gstart[gsorted]
    pos = tile_col[blk, halfv] * BLK + rank  # padded slot within the core
    idxval = np.where(halfv == 1, src - half, src).astype(np.int16)

    total_pad = T_total * BLK
    deg = np.bincount(dst, minlength=n_nodes).astype(np.float32)
    rdeg = 1.0 / np.maximum(deg, 1.0)
    per_core = []
    for c in range(n_cores):
        m = core == c
        idx_pad = np.zeros(total_pad, np.int16)
        rel_pad = np.full(total_pad, SENT, np.float32)
        idx_pad[pos[m]] = idxval[m]
        rel_pad[pos[m]] = rel[m]
        # dma_gather index layout: partition e%16, column e//16, replicated
        # across the eight 16-partition groups.
        idx_mat = np.ascontiguousarray(np.tile(idx_pad.reshape(-1, 16).T, (8, 1)))
        # gather output layout: partition e%128, tile-column e//128.
        rel_mat = np.ascontiguousarray(rel_pad.reshape(-1, BLK).T)
        xT = np.zeros((F, NB * BLK), np.float32)
        xT[:, :NP] = x[c * NP : (c + 1) * NP].T
        # 1/max(in-degree,1) for this core's dsts, replicated on all 128
        # partitions (columns = dst slots) for the aggT elementwise multiply.
        rc = np.zeros(NB * BLK, np.float32)
        rc[:NP] = rdeg[c * NP : (c + 1) * NP]
        rcnt_mat = np.ascontiguousarray(np.broadcast_to(rc, (128, NB * BLK)))
        per_core.append((idx_mat, rel_mat, xT, rcnt_mat))
    return caps, chunks, T_total, NP, NB, per_core


def _build_program(caps, chunks, T_total, NP, NB, n_nodes, half, ablate=(),
                   repeat=1, nq=N_QUEUES, gmax_idx=GMAX_IDX,
                   scratch=DMA_SCRATCH):
    dt = mybir.dt
    mdt = dt.bfloat16 if MSG_BF16 else dt.float32
    nc = bacc.Bacc(
        "TRN2", target_bir_lowering=False, debug=False,
        dynamic_dma_scratch_size=scratch,
        num_swdge_queues=nq,
    )

    x_d = nc.dram_tensor("x", [n_nodes, F], mdt, kind="ExternalInput")
    xT_d = nc.dram_tensor("xT", [F, NB * BLK], dt.float32, kind="ExternalInput")
    idx_d = nc.dram_tensor("idx", [128, T_total * 8], dt.int16, kind="ExternalInput")
    rel_d = nc.dram_tensor("rel", [128, T_total], dt.float32, kind="ExternalInput")
    wlT_d = nc.dram_tensor("wlT", [F, F], dt.float32, kind="ExternalInput")
    wrT_d = nc.dram_tensor("wrT", [F, F], dt.float32, kind="ExternalInput")
    bias_d = nc.dram_tensor("bias", [1, F], dt.float32, kind="ExternalInput")
    iota_d = nc.dram_tensor("iota", [128, 128], mdt, kind="ExternalInput")
    # 1/max(in-degree,1) replicated on all 128 partitions: column = dst slot.
    rcnt_d = nc.dram_tensor("rcnt", [128, NB * BLK], dt.float32, kind="ExternalInput")
    y_d = nc.dram_tensor("y", [NP, F], dt.float32, kind="ExternalOutput")

    TCMAX = max(ch["lo_tiles"] + ch["hi_tiles"] for ch in chunks)

    with tile.TileContext(nc) as tc, ExitStack() as ctx:
        res = ctx.enter_context(tc.tile_pool(name="res", bufs=1))
        msgs_p = ctx.enter_context(tc.tile_pool(name="msgs", bufs=MSGS_BUFS))
        s_p = ctx.enter_context(tc.tile_pool(name="sel", bufs=4))
        work_p = ctx.enter_context(tc.tile_pool(name="work", bufs=3))
        small_p = ctx.enter_context(tc.tile_pool(name="small", bufs=4))
        agg_p = ctx.enter_context(tc.tile_pool(name="agg", bufs=3, space="PSUM"))
        po_p = ctx.enter_context(tc.tile_pool(name="po", bufs=2, space="PSUM"))

        xT_sb = res.tile([F, NB * BLK], dt.float32)
        nc.sync.dma_start(xT_sb[:], xT_d[:])
        rel_sb = res.tile([128, T_total], dt.float32)
        nc.sync.dma_start(rel_sb[:], rel_d[:])
        idx_sb = res.tile([128, T_total * 8], dt.int16)
        nc.sync.dma_start(idx_sb[:], idx_d[:])
        iota_sb = res.tile([128, 128], mdt)
        nc.sync.dma_start(iota_sb[:], iota_d[:])
        rcnt_sb = res.tile([128, NB * BLK], dt.float32)
        nc.sync.dma_start(rcnt_sb[:], rcnt_d[:])
        wlT_sb = res.tile([F, F], dt.float32)
        nc.sync.dma_start(wlT_sb[:], wlT_d[:])
        wrT_sb = res.tile([F, F], dt.float32)
        nc.sync.dma_start(wrT_sb[:], wrT_d[:])
        bias_sb = res.tile([1, F], dt.float32)
        nc.sync.dma_start(bias_sb[:], bias_d[:])
        onesrow_sb = res.tile([1, 128], dt.float32)
        nc.vector.memset(onesrow_sb[:], 1.0)
        probe_id = None
        if "oldagg" in ablate:
            probe_id = res.tile([128, 128], dt.float32)
            nc.vector.memset(probe_id[:], 0.0)

        qc = [0]  # round-robin gather queue counter
        yt_p = ctx.enter_context(tc.tile_pool(name="yt", bufs=2))
        for _rep in range(repeat):
          for ch in chunks:
            msgs = msgs_p.tile([128, TCMAX, F], mdt, tag="msgs")
            yt = None
            if BATCH_OUT:
                yt = yt_p.tile([128, len(ch["blocks"]), F], dt.float32, tag="yt")
            lo_t, hi_t = ch["lo_tiles"], ch["hi_tiles"]
            st = ch["start"]
            # Ring-capacity limit on indices per dma_gather call.
            GMAX = gmax_idx // BLK
            for base, count, src_ap in (
                (0, lo_t, x_d[0:half, :]),
                (lo_t, hi_t, x_d[half:n_nodes, :]),
            ):
                for g0 in range(0, count, GMAX):
                    gt = min(GMAX, count - g0)
                    if "gsmall" in ablate:
                        gt = 1  # timing-only ablation: 1/8 gather bytes
                    s0 = base + g0  # slot within chunk
                    nc.gpsimd.dma_gather(
                        out_ap=msgs[:, s0 : s0 + gt, :],
                        in_ap=src_ap,
                        idxs_ap=idx_sb[:, (st + s0) * 8 : (st + s0 + gt) * 8],
                        num_idxs=gt * BLK,
                        num_idxs_reg=gt * BLK,
                        elem_size=F,
                        queue_num=qc[0] % nq,
                        single_packet=SINGLE_PACKET,
                    )
                    qc[0] += 1
            for bi, b in enumerate(ch["blocks"]):
                slots = []
                for h in (0, 1):
                    if caps[b, h]:
                        g0 = ch["gcols"][(b, h)]
                        slots.extend(range(g0 - st, g0 - st + int(caps[b, h])))
                nb = min(BLK, NP - b * BLK)
                psum_agg = agg_p.tile([128, F], dt.float32, tag="agg")
                if not slots or "seg" in ablate:
                    nc.vector.memset(psum_agg[:], 0.0)
                for j, slot in enumerate(slots):
                    if "seg" in ablate:
                        break
                    tcol = st + slot
                    S = s_p.tile([128, 128], mdt, tag="S")
                    if "sbuild" in ablate:
                        S = iota_sb
                    elif (tcol % ACT_SBUILD[1]) < ACT_SBUILD[0]:
                        # exact one-hot on ACT: |rel - iota| then relu(1 - | . |)
                        t1 = s_p.tile([128, 128], mdt, tag="Sa")
                        nc.scalar.activation(
                            out=t1[:], in_=iota_sb[:],
                            func=mybir.ActivationFunctionType.Abs,
                            bias=rel_sb[:, tcol : tcol + 1], scale=-1.0,
                        )
                        nc.scalar.activation(
                            out=S[:], in_=t1[:],
                            func=mybir.ActivationFunctionType.Relu,
                            bias=1.0, scale=-1.0,
                        )
                    else:
                        nc.vector.tensor_scalar(
                            out=S[:],
                            in0=iota_sb[:],
                            scalar1=rel_sb[:, tcol : tcol + 1],
                            scalar2=None,
                            op0=mybir.AluOpType.is_equal,
                        )
                    first, last = j == 0, j == len(slots) - 1
                    if "mm2" not in ablate:
                        # aggT[f, d] = sum_e msgs[e, f] * S[e, d]: transposed
                        # aggregate, so no PE transpose is needed before the
                        # output GEMM (mean.T feeds lhsT directly).
                        if "oldagg" in ablate:
                            nc.tensor.matmul(
                                psum_agg[:], lhsT=S[:], rhs=msgs[:, slot, :],
                                start=first, stop=last,
                            )
                        else:
                            nc.tensor.matmul(
                                psum_agg[:], lhsT=msgs[:, slot, :], rhs=S[:],
                                start=first, stop=last,
                            )
                    elif first:
                        nc.vector.memset(psum_agg[:], 0.0)
                mT = work_p.tile([128, 128], dt.float32, tag="mT")
                if "oldagg" in ablate:
                    # timing-only probe of the pre-aggT pipeline: per-dst
                    # scale (wrong values), PE transpose, PSUM->SBUF copy.
                    mean = work_p.tile([128, F], dt.float32, tag="mean")
                    nc.vector.tensor_scalar_mul(
                        mean[:], psum_agg[:], rcnt_sb[:, b : b + 1]
                    )
                    psum_t = po_p.tile([128, 128], dt.float32, tag="pt")
                    nc.tensor.transpose(psum_t[:], mean[:], probe_id[:])
                    nc.vector.tensor_copy(out=mT[:], in_=psum_t[:])
                else:
                    nc.vector.tensor_mul(
                        mT[:], psum_agg[:], rcnt_sb[:, b * BLK : (b + 1) * BLK]
                    )
                psum_o = po_p.tile([128, F], dt.float32, tag="po")
                nc.tensor.matmul(
                    psum_o[:], lhsT=mT[:], rhs=wlT_sb[:], start=True, stop=False
                )
                nc.tensor.matmul(
                    psum_o[:], lhsT=xT_sb[:, b * BLK : (b + 1) * BLK], rhs=wrT_sb[:],
                    start=False, stop=False,
                )
                nc.tensor.matmul(
                    psum_o[:], lhsT=onesrow_sb[0:1, :], rhs=bias_sb[0:1, :],
                    start=False, stop=True,
                )
                sq = work_p.tile([128, F], dt.float32, tag="sq")
                ss = small_p.tile([128, 1], dt.float32, tag="ss")
                nc.scalar.activation(
                    out=sq[:], in_=psum_o[:],
                    func=mybir.ActivationFunctionType.Square, accum_out=ss[:],
                )
                ssm = small_p.tile([128, 1], dt.float32, tag="ssm")
                nc.vector.tensor_scalar_max(ssm[:], ss[:], 1e-24)
                nrm = small_p.tile([128, 1], dt.float32, tag="nrm")
                nc.scalar.sqrt(nrm[:], ssm[:])
                rn = small_p.tile([128, 1], dt.float32, tag="rn")
                nc.vector.reciprocal(rn[:], nrm[:])
                if BATCH_OUT:
                    nc.vector.tensor_scalar_mul(
                        yt[:, bi, :], psum_o[:], rn[:, 0:1]
                    )
                    if nb < BLK:  # partial last block: narrow row-DMA
                        nc.sync.dma_start(
                            y_d[b * BLK : b * BLK + nb, :], yt[0:nb, bi, :]
                        )
                else:
                    outt = work_p.tile([128, F], dt.float32, tag="outt")
                    nc.vector.tensor_scalar_mul(outt[:], psum_o[:], rn[:, 0:1])
                    nc.sync.dma_start(
                        y_d[b * BLK : b * BLK + nb, :], outt[0:nb, :]
                    )
            if BATCH_OUT:
                full = [b for b in ch["blocks"] if (b + 1) * BLK <= NP]
                if full:
                    b0, nf = full[0], len(full)
                    nc.sync.dma_start(
                        y_d[b0 * BLK : (b0 + nf) * BLK, :].rearrange(
                            "(nb p) f -> p nb f", p=BLK
                        ),
                        yt[:, 0:nf, :],
                    )

    nc.compile()
    return nc


_CACHE = {}


def _prepare(inputs, n_nodes=N_NODES, n_cores=N_CORES, half=HALF,
             chunk_blocks=CHUNK_BLOCKS):
    import ml_dtypes
    mnp = ml_dtypes.bfloat16 if MSG_BF16 else np.float32
    x = np.asarray(inputs["x"], np.float32)
    ei = np.asarray(inputs["edge_index"], np.int64)
    W_l = np.asarray(inputs["W_l"], np.float32)
    b_l = np.asarray(inputs["b_l"], np.float32)
    W_r = np.asarray(inputs["W_r"], np.float32)
    src, dst = ei[0], ei[1]

    caps, chunks, T_total, NP, NB, per_core = _pack_inputs(
        x, src, dst, n_nodes, n_cores, half, chunk_blocks
    )
    key = (n_nodes, n_cores, half, chunk_blocks, caps.tobytes())
    nc = _CACHE.get(key)
    if nc is None:
        nc = _build_program(caps, chunks, T_total, NP, NB, n_nodes, half)
        _CACHE[key] = nc

    iota = np.ascontiguousarray(
        np.broadcast_to(np.arange(128), (128, 128)).astype(mnp)
    )
    x_m = np.ascontiguousarray(x.astype(mnp))
    wlT = np.ascontiguousarray(W_l.T)
    wrT = np.ascontiguousarray(W_r.T)
    bias = np.ascontiguousarray(b_l[None, :])
    in_maps = []
    for c in range(n_cores):
        idx_mat, rel_mat, xT, rcnt_mat = per_core[c]
        in_maps.append(
            {
                "x": x_m, "xT": xT, "idx": idx_mat, "rel": rel_mat,
                "wlT": wlT, "wrT": wrT, "bias": bias, "iota": iota,
                "rcnt": rcnt_mat,
            }
        )
    return nc, in_maps, NP


def _run(inputs, trace=False):
    nc, in_maps, NP = _prepare(inputs)
    r = run_bass_kernel_spmd(nc, in_maps, list(range(N_CORES)), trace=trace)
    y = np.concatenate([r.results[c]["y"] for c in range(N_CORES)], axis=0)
    return y, r


def kernel(**inputs) -> np.ndarray:
    y, _ = _run(inputs)
    return y



# revision 33
# speedup vs baseline: 4.5413x; 1.0690x over previous
# GraphSAGE mean-aggregation layer on 8 Trainium2 NeuronCores.
#
# Sharding: destination nodes are partitioned across the 8 cores (6250 each).
# Each core receives the full node-feature table x (for gathers), plus
# host-packed per-core metadata:
#   - its edges, grouped by 128-destination "blocks", padded to 128-edge tiles
#   - int16 gather indices (dma_gather requires int16, so edges are split into
#     src<32768 ("lo") and src>=32768 ("hi") groups gathered from two base
#     offsets of x)
#   - per-edge destination-within-block ids (f32, sentinel 999 for padding)
# Device per block of 128 destinations:
#   aggT[feat, dst] = sum over edge tiles of msgs_t.T @ S_t   (PE, PSUM accum)
#     where S_t[e, d] = (rel[e] == d) is built on DVE/ACT via is_equal vs an
#     iota row; padded edges have rel=SENT so they contribute nothing.
#   meanT = aggT * rcnt (1/max(in-degree,1), host-precomputed) (DVE)
#   out  = meanT.T @ W_l.T + x_dst @ W_r.T + b_l               (PE; meanT is
#     already the lhsT layout, x_dst supplied pre-transposed from host)
#   y    = out / max(||out||_2, 1e-12)                         (ACT+DVE)
# Gathers are issued round-robin on 4 SWDGE queues (descriptor generation for
# queue q runs on Q7 core pair 2q/2q+1), pipelining desc-gen across core pairs.
#
# The Bass program is identical across cores (capacities = max over cores),
# so it runs SPMD via run_bass_kernel_spmd; per-core data differs only in the
# input tensors.

import os
from contextlib import ExitStack

import numpy as np

import concourse.bacc as bacc
import concourse.mybir as mybir
import concourse.tile as tile
from concourse.bass_utils import run_bass_kernel_spmd

F = 128          # feature dim (in_c == out_c == 128)
BLK = 128        # destinations per block (= PSUM partition dim)
N_NODES = 50000
N_EDGES = 800000
N_CORES = 8
HALF = 32768     # int16 gather-index limit
CHUNK_BLOCKS = 12  # dst blocks per gather chunk
ACT_SBUILD = (0, 1)  # build S on ACT for this fraction of tiles (all-DVE wins)
MSG_BF16 = True      # gather/aggregate messages in bf16 (GEMMs stay f32)
SRC_SORT = False     # sort bucket edges by src (clusters HBM reads; slower)
MSGS_BUFS = 2        # msgs tile double buffering
SINGLE_PACKET = False  # per-descriptor packets interleave queues better
BATCH_OUT = False    # per-block output DMAs beat the strided chunk DMA
SEL_BUFS = 6         # S one-hot tile pool depth
AGG_BUFS = 4         # PSUM aggT pool depth
PO_BUFS = 3          # PSUM output pool depth
DMA_SCRATCH = 16384  # SWDGE descriptor-ring bytes/partition
GMAX_IDX = 1280      # indices per dma_gather call (>1280 hangs: ring capacity)
N_QUEUES = 4         # SWDGE queues; queue q's descriptors are generated on Q7
                     # core pair 2q/2q+1, so round-robin parallelizes desc-gen
SENT = 512.0         # padded-edge rel sentinel (exact in bf16, >= 128)


def _make_plan(counts_max, chunk_blocks):
    """Shared (across cores) tile schedule from per-(block,half) max counts."""
    caps = -(-counts_max // BLK)  # ceil div -> tiles per (block, half)
    NB = caps.shape[0]
    chunks = []
    t0 = 0
    for k0 in range(0, NB, chunk_blocks):
        bl = list(range(k0, min(k0 + chunk_blocks, NB)))
        gcols = {}
        t = t0
        for h in (0, 1):
            for b in bl:
                if caps[b, h]:
                    gcols[(b, h)] = t
                    t += int(caps[b, h])
        chunks.append(
            dict(
                blocks=bl,
                gcols=gcols,
                start=t0,
                lo_tiles=int(sum(caps[b, 0] for b in bl)),
                hi_tiles=int(sum(caps[b, 1] for b in bl)),
            )
        )
        t0 = t
    return caps, chunks, t0


def _pack_inputs(x, src, dst, n_nodes, n_cores, half, chunk_blocks):
    """Host-side graph partitioning: bucket edges by (core, block, half),
    pad each bucket to whole 128-edge tiles, emit per-core device arrays."""
    NP = n_nodes // n_cores
    assert NP * n_cores == n_nodes
    NB = -(-NP // BLK)
    core = dst // NP
    ldst = dst - core * NP
    blk = ldst // BLK
    rel = (ldst - blk * BLK).astype(np.float32)
    halfv = (src >= half).astype(np.int64)
    gkey = (core * NB + blk) * 2 + halfv
    counts = np.bincount(gkey, minlength=n_cores * NB * 2).reshape(n_cores, NB, 2)
    caps, chunks, T_total = _make_plan(counts.max(axis=0), chunk_blocks)

    tile_col = np.zeros((NB, 2), np.int64)
    for ch in chunks:
        for (b, h), c in ch["gcols"].items():
            tile_col[b, h] = c

    # SRC_SORT orders each bucket's edges by src (ascending HBM addresses per
    # dma_gather call). Measured: clustering HURTS SDMA throughput on random
    # graphs, so default off.
    order = np.lexsort((src, gkey)) if SRC_SORT else np.argsort(gkey, kind="stable")
    gsorted = gkey[order]
    gstart = np.searchsorted(gsorted, np.arange(n_cores * NB * 2))
    rank = np.empty(len(gkey), np.int64)
    rank[order] = np.arange(len(gkey)) - gstart[gsorted]
    pos = tile_col[blk, halfv] * BLK + rank  # padded slot within the core
    idxval = np.where(halfv == 1, src - half, src).astype(np.int16)

    total_pad = T_total * BLK
    deg = np.bincount(dst, minlength=n_nodes).astype(np.float32)
    rdeg = 1.0 / np.maximum(deg, 1.0)
    per_core = []
    for c in range(n_cores):
        m = core == c
        idx_pad = np.zeros(total_pad, np.int16)
        rel_pad = np.full(total_pad, SENT, np.float32)
        idx_pad[pos[m]] = idxval[m]
        rel_pad[pos[m]] = rel[m]
        # dma_gather index layout: partition e%16, column e//16, replicated
        # across the eight 16-partition groups.
        idx_mat = np.ascontiguousarray(np.tile(idx_pad.reshape(-1, 16).T, (8, 1)))
        # gather output layout: partition e%128, tile-column e//128.
        rel_mat = np.ascontiguousarray(rel_pad.reshape(-1, BLK).T)
        xT = np.zeros((F, NB * BLK), np.float32)
        xT[:, :NP] = x[c * NP : (c + 1) * NP].T
        # 1/max(in-degree,1) for this core's dsts, replicated on all 128
        # partitions (columns = dst slots) for the aggT elementwise multiply.
        rc = np.zeros(NB * BLK, np.float32)
        rc[:NP] = rdeg[c * NP : (c + 1) * NP]
        rcnt_mat = np.ascontiguousarray(np.broadcast_to(rc, (128, NB * BLK)))
        per_core.append((idx_mat, rel_mat, xT, rcnt_mat))
    return caps, chunks, T_total, NP, NB, per_core


def _build_program(caps, chunks, T_total, NP, NB, n_nodes, half, ablate=(),
                   repeat=1, nq=N_QUEUES, gmax_idx=GMAX_IDX,
                   scratch=DMA_SCRATCH):
    dt = mybir.dt
    mdt = dt.bfloat16 if MSG_BF16 else dt.float32
    nc = bacc.Bacc(
        "TRN2", target_bir_lowering=False, debug=False,
        dynamic_dma_scratch_size=scratch,
        num_swdge_queues=nq,
    )

    x_d = nc.dram_tensor("x", [n_nodes, F], mdt, kind="ExternalInput")
    xT_d = nc.dram_tensor("xT", [F, NB * BLK], dt.float32, kind="ExternalInput")
    idx_d = nc.dram_tensor("idx", [128, T_total * 8], dt.int16, kind="ExternalInput")
    rel_d = nc.dram_tensor("rel", [128, T_total], dt.float32, kind="ExternalInput")
    wlT_d = nc.dram_tensor("wlT", [F, F], dt.float32, kind="ExternalInput")
    wrT_d = nc.dram_tensor("wrT", [F, F], dt.float32, kind="ExternalInput")
    bias_d = nc.dram_tensor("bias", [1, F], dt.float32, kind="ExternalInput")
    iota_d = nc.dram_tensor("iota", [128, 128], mdt, kind="ExternalInput")
    # 1/max(in-degree,1) replicated on all 128 partitions: column = dst slot.
    rcnt_d = nc.dram_tensor("rcnt", [128, NB * BLK], dt.float32, kind="ExternalInput")
    y_d = nc.dram_tensor("y", [NP, F], dt.float32, kind="ExternalOutput")

    TCMAX = max(ch["lo_tiles"] + ch["hi_tiles"] for ch in chunks)

    with tile.TileContext(nc) as tc, ExitStack() as ctx:
        res = ctx.enter_context(tc.tile_pool(name="res", bufs=1))
        msgs_p = ctx.enter_context(tc.tile_pool(name="msgs", bufs=MSGS_BUFS))
        s_p = ctx.enter_context(tc.tile_pool(name="sel", bufs=SEL_BUFS))
        work_p = ctx.enter_context(tc.tile_pool(name="work", bufs=3))
        small_p = ctx.enter_context(tc.tile_pool(name="small", bufs=4))
        agg_p = ctx.enter_context(
            tc.tile_pool(name="agg", bufs=AGG_BUFS, space="PSUM"))
        po_p = ctx.enter_context(
            tc.tile_pool(name="po", bufs=PO_BUFS, space="PSUM"))

        xT_sb = res.tile([F, NB * BLK], dt.float32)
        nc.sync.dma_start(xT_sb[:], xT_d[:])
        rel_sb = res.tile([128, T_total], dt.float32)
        nc.sync.dma_start(rel_sb[:], rel_d[:])
        idx_sb = res.tile([128, T_total * 8], dt.int16)
        nc.sync.dma_start(idx_sb[:], idx_d[:])
        iota_sb = res.tile([128, 128], mdt)
        nc.sync.dma_start(iota_sb[:], iota_d[:])
        rcnt_sb = res.tile([128, NB * BLK], dt.float32)
        nc.sync.dma_start(rcnt_sb[:], rcnt_d[:])
        wlT_sb = res.tile([F, F], dt.float32)
        nc.sync.dma_start(wlT_sb[:], wlT_d[:])
        wrT_sb = res.tile([F, F], dt.float32)
        nc.sync.dma_start(wrT_sb[:], wrT_d[:])
        bias_sb = res.tile([1, F], dt.float32)
        nc.sync.dma_start(bias_sb[:], bias_d[:])
        onesrow_sb = res.tile([1, 128], dt.float32)
        nc.vector.memset(onesrow_sb[:], 1.0)
        probe_id = None
        if "oldagg" in ablate:
            probe_id = res.tile([128, 128], dt.float32)
            nc.vector.memset(probe_id[:], 0.0)

        qc = [0]  # round-robin gather queue counter
        yt_p = ctx.enter_context(tc.tile_pool(name="yt", bufs=2))
        for _rep in range(repeat):
          for ch in chunks:
            msgs = msgs_p.tile([128, TCMAX, F], mdt, tag="msgs")
            yt = None
            if BATCH_OUT:
                yt = yt_p.tile([128, len(ch["blocks"]), F], dt.float32, tag="yt")
            lo_t, hi_t = ch["lo_tiles"], ch["hi_tiles"]
            st = ch["start"]
            # Ring-capacity limit on indices per dma_gather call.
            GMAX = gmax_idx // BLK
            for base, count, src_ap in (
                (0, lo_t, x_d[0:half, :]),
                (lo_t, hi_t, x_d[half:n_nodes, :]),
            ):
                for g0 in range(0, count, GMAX):
                    gt = min(GMAX, count - g0)
                    if "gsmall" in ablate:
                        gt = 1  # timing-only ablation: 1/8 gather bytes
                    s0 = base + g0  # slot within chunk
                    nc.gpsimd.dma_gather(
                        out_ap=msgs[:, s0 : s0 + gt, :],
                        in_ap=src_ap,
                        idxs_ap=idx_sb[:, (st + s0) * 8 : (st + s0 + gt) * 8],
                        num_idxs=gt * BLK,
                        num_idxs_reg=gt * BLK,
                        elem_size=F,
                        queue_num=qc[0] % nq,
                        single_packet=SINGLE_PACKET,
                    )
                    qc[0] += 1
            for bi, b in enumerate(ch["blocks"]):
                slots = []
                for h in (0, 1):
                    if caps[b, h]:
                        g0 = ch["gcols"][(b, h)]
                        slots.extend(range(g0 - st, g0 - st + int(caps[b, h])))
                nb = min(BLK, NP - b * BLK)
                psum_agg = agg_p.tile([128, F], dt.float32, tag="agg")
                if not slots or "seg" in ablate:
                    nc.vector.memset(psum_agg[:], 0.0)
                for j, slot in enumerate(slots):
                    if "seg" in ablate:
                        break
                    tcol = st + slot
                    S = s_p.tile([128, 128], mdt, tag="S")
                    if "sbuild" in ablate:
                        S = iota_sb
                    elif (tcol % ACT_SBUILD[1]) < ACT_SBUILD[0]:
                        # exact one-hot on ACT: |rel - iota| then relu(1 - | . |)
                        t1 = s_p.tile([128, 128], mdt, tag="Sa")
                        nc.scalar.activation(
                            out=t1[:], in_=iota_sb[:],
                            func=mybir.ActivationFunctionType.Abs,
                            bias=rel_sb[:, tcol : tcol + 1], scale=-1.0,
                        )
                        nc.scalar.activation(
                            out=S[:], in_=t1[:],
                            func=mybir.ActivationFunctionType.Relu,
                            bias=1.0, scale=-1.0,
                        )
                    else:
                        nc.vector.tensor_scalar(
                            out=S[:],
                            in0=iota_sb[:],
                            scalar1=rel_sb[:, tcol : tcol + 1],
                            scalar2=None,
                            op0=mybir.AluOpType.is_equal,
                        )
                    first, last = j == 0, j == len(slots) - 1
                    if "mm2" not in ablate:
                        # aggT[f, d] = sum_e msgs[e, f] * S[e, d]: transposed
                        # aggregate, so no PE transpose is needed before the
                        # output GEMM (mean.T feeds lhsT directly).
                        if "oldagg" in ablate:
                            nc.tensor.matmul(
                                psum_agg[:], lhsT=S[:], rhs=msgs[:, slot, :],
                                start=first, stop=last,
                            )
                        else:
                            nc.tensor.matmul(
                                psum_agg[:], lhsT=msgs[:, slot, :], rhs=S[:],
                                start=first, stop=last,
                            )
                    elif first:
                        nc.vector.memset(psum_agg[:], 0.0)
                mT = work_p.tile([128, 128], dt.float32, tag="mT")
                if "oldagg" in ablate:
                    # timing-only probe of the pre-aggT pipeline: per-dst
                    # scale (wrong values), PE transpose, PSUM->SBUF copy.
                    mean = work_p.tile([128, F], dt.float32, tag="mean")
                    nc.vector.tensor_scalar_mul(
                        mean[:], psum_agg[:], rcnt_sb[:, b : b + 1]
                    )
                    psum_t = po_p.tile([128, 128], dt.float32, tag="pt")
                    nc.tensor.transpose(psum_t[:], mean[:], probe_id[:])
                    nc.vector.tensor_copy(out=mT[:], in_=psum_t[:])
                else:
                    nc.vector.tensor_mul(
                        mT[:], psum_agg[:], rcnt_sb[:, b * BLK : (b + 1) * BLK]
                    )
                psum_o = po_p.tile([128, F], dt.float32, tag="po")
                nc.tensor.matmul(
                    psum_o[:], lhsT=mT[:], rhs=wlT_sb[:], start=True, stop=False
                )
                nc.tensor.matmul(
                    psum_o[:], lhsT=xT_sb[:, b * BLK : (b + 1) * BLK], rhs=wrT_sb[:],
                    start=False, stop=False,
                )
                nc.tensor.matmul(
                    psum_o[:], lhsT=onesrow_sb[0:1, :], rhs=bias_sb[0:1, :],
                    start=False, stop=True,
                )
                sq = work_p.tile([128, F], dt.float32, tag="sq")
                ss = small_p.tile([128, 1], dt.float32, tag="ss")
                nc.scalar.activation(
                    out=sq[:], in_=psum_o[:],
                    func=mybir.ActivationFunctionType.Square, accum_out=ss[:],
                )
                ssm = small_p.tile([128, 1], dt.float32, tag="ssm")
                nc.vector.tensor_scalar_max(ssm[:], ss[:], 1e-24)
                nrm = small_p.tile([128, 1], dt.float32, tag="nrm")
                nc.scalar.sqrt(nrm[:], ssm[:])
                rn = small_p.tile([128, 1], dt.float32, tag="rn")
                nc.vector.reciprocal(rn[:], nrm[:])
                if BATCH_OUT:
                    nc.vector.tensor_scalar_mul(
                        yt[:, bi, :], psum_o[:], rn[:, 0:1]
                    )
                    if nb < BLK:  # partial last block: narrow row-DMA
                        nc.sync.dma_start(
                            y_d[b * BLK : b * BLK + nb, :], yt[0:nb, bi, :]
                        )
                else:
                    outt = work_p.tile([128, F], dt.float32, tag="outt")
                    nc.vector.tensor_scalar_mul(outt[:], psum_o[:], rn[:, 0:1])
                    nc.sync.dma_start(
                        y_d[b * BLK : b * BLK + nb, :], outt[0:nb, :]
                    )
            if BATCH_OUT:
                full = [b for b in ch["blocks"] if (b + 1) * BLK <= NP]
                if full:
                    b0, nf = full[0], len(full)
                    nc.sync.dma_start(
                        y_d[b0 * BLK : (b0 + nf) * BLK, :].rearrange(
                            "(nb p) f -> p nb f", p=BLK
                        ),
                        yt[:, 0:nf, :],
                    )

    nc.compile()
    return nc


_CACHE = {}


def _prepare(inputs, n_nodes=N_NODES, n_cores=N_CORES, half=HALF,
             chunk_blocks=CHUNK_BLOCKS):
    import ml_dtypes
    mnp = ml_dtypes.bfloat16 if MSG_BF16 else np.float32
    x = np.asarray(inputs["x"], np.float32)
    ei = np.asarray(inputs["edge_index"], np.int64)
    W_l = np.asarray(inputs["W_l"], np.float32)
    b_l = np.asarray(inputs["b_l"], np.float32)
    W_r = np.asarray(inputs["W_r"], np.float32)
    src, dst = ei[0], ei[1]

    caps, chunks, T_total, NP, NB, per_core = _pack_inputs(
        x, src, dst, n_nodes, n_cores, half, chunk_blocks
    )
    key = (n_nodes, n_cores, half, chunk_blocks, caps.tobytes())
    nc = _CACHE.get(key)
    if nc is None:
        nc = _build_program(caps, chunks, T_total, NP, NB, n_nodes, half)
        _CACHE[key] = nc

    iota = np.ascontiguousarray(
        np.broadcast_to(np.arange(128), (128, 128)).astype(mnp)
    )
    x_m = np.ascontiguousarray(x.astype(mnp))
    wlT = np.ascontiguousarray(W_l.T)
    wrT = np.ascontiguousarray(W_r.T)
    bias = np.ascontiguousarray(b_l[None, :])
    in_maps = []
    for c in range(n_cores):
        idx_mat, rel_mat, xT, rcnt_mat = per_core[c]
        in_maps.append(
            {
                "x": x_m, "xT": xT, "idx": idx_mat, "rel": rel_mat,
                "wlT": wlT, "wrT": wrT, "bias": bias, "iota": iota,
                "rcnt": rcnt_mat,
            }
        )
    return nc, in_maps, NP


def _run(inputs, trace=False):
    nc, in_maps, NP = _prepare(inputs)
    r = run_bass_kernel_spmd(nc, in_maps, list(range(N_CORES)), trace=trace)
    y = np.concatenate([r.results[c]["y"] for c in range(N_CORES)], axis=0)
    return y, r


def kernel(**inputs) -> np.ndarray:
    y, _ = _run(inputs)
    return y

